# revision 24
# baseline (speedup 1.0000x reference)
import sys

import numpy as np
import ml_dtypes

sys.path.insert(0, "/opt/trn_rl_repo")

# NT-Xent contrastive loss over emb_cat [8192, 256] f32, T=0.5.
#   z = row-normalize(emb); sim = z @ z.T
#   denom_i = sum_{j != i} exp(sim_ij / T); pos_i = sim_{i, (i+4096) mod 8192}
#   loss = sum_i (ln(denom_i) - pos_i / T) / 4096
#
# v5e layout: the O(N*D) prep (normalize, scale by 1/sqrt(T), transpose,
# fp8e4 cast) runs on the host (like the host-side roll/log-combine of
# earlier versions); each core receives w-transposed fp8 blocks for its
# 5 rotated column groups in DoubleRow layout.  The device computes the
# O(N^2) part: sim tiles via fp8 DoubleRow matmuls, exp, per-row sums,
# and per-column sums shipped so peer cores can complete their missing
# symmetric halves (5/8 symmetric-halving as v3).  Host combines in f64
# and computes positives exactly from z.
#
# Work cuts vs the plain 5/8 scheme (33280 of 40960 col-cycles, -19%):
# - g0 (diagonal, symmetric): row tile m computes only cols [m*128,1024)
#   (upper triangle); missing cols come from triangle colsum chains.
# - g4 (pair block, both c and c+4 compute it): core c computes cols
#   [0,512) for m<4, [512,1024) for m>=4; host swaps the pair block's
#   column halves for c>=4 so the pair covers all quadrants once;
#   missing half-rowsums come from the partner's cs4a/cs4b chains.
#
# exp runs on BOTH fast psum-readers (gpsimd has no PSUM port):
#   'A': ACT native Exp (psum -> bf16 sbuf) + accum_out rowsum
#   'V': DVE Schraudolph: i16 = rint(x*128*log2e + 16250.5) bitcast bf16
#        ~= exp(x) (mean err ~1e-4 here), + DVE rowsum reduce
# Colsums via PE ones-matmul chains into psum (partitions 0/32/64 pack 3
# chunks per tile), staged to SBUF by ACT/DVE copies (DMA cannot read
# PSUM), then DMA'd out.  PSUM: mt pool 3 x [128,1024] (6 banks) + 2
# live colsum tiles = 8 banks; the 3-deep mt ring keeps PE/ACT/DVE
# pipelined (a 2-deep ring of wider tiles measured ~10us slower).

N = 8192
D = 256
B = 4096
NCORES = 8
LOCAL = N // NCORES
T = 0.5
S_EXP = 184.6628           # 128 * log2(e): bf16 Schraudolph scale
B_EXP = 16250.5            # 127*128 + sigma, sigma=-5.5 zeroes mean err


# Greedy ACT/DVE balance, constants fitted from measured traces:
# ACT activate ~0.833*w+260 + 283 accum-read + sems; DVE schraudolph
# ~0.94*w + reduce ~0.97*w (TENSOR_REDUCE gets no 2x mode) + overheads.
# The RESULTING PER-POSITION PATTERN is load-bearing, not just the
# aggregate split: cV=1.93 beats 1.91 by ~1us purely by making blk4's
# tail strictly A/V-alternating; a 15-position repack (cV=2.10) cost
# +9us, moving V tiles into early blk0 (tV0=2000) cost +3.8us (the wide
# ACT strips there shield pipeline fill), and a 2-position blk3 swap
# (cV=1.96) cost +1us.  This cV=1.93 pattern is the measured optimum.
def _assign_engines(widths):
    tA = 0.0
    tV = 2900.0   # psum->sbuf colsum staging copies ride on DVE
    out = []
    for wdt in widths:
        cA = wdt * 0.833 + 750
        cV = wdt * 1.93 + 500
        if tA + cA <= tV + cV:
            out.append("A")
            tA += cA
        else:
            out.append("V")
            tV += cV
    return out


_NC_CACHE = {}


def _build_program():
    from concourse import bacc, mybir, tile

    nc = bacc.Bacc("TRN2", target_bir_lowering=False, debug=False)
    f32 = mybir.dt.float32
    bf16 = mybir.dt.bfloat16
    f8 = mybir.dt.float8e4
    i16 = mybir.dt.int16
    AF = mybir.ActivationFunctionType
    ALU = mybir.AluOpType
    AX = mybir.AxisListType
    PM = mybir.MatmulPerfMode

    wt0 = nc.dram_tensor("wt0", (128, 2, LOCAL), f8,
                         kind="ExternalInput").ap()
    wt14 = nc.dram_tensor("wt14", (128, 4, 2, LOCAL), f8,
                          kind="ExternalInput").ap()
    outd = nc.dram_tensor("acc", (128, 40), f32, kind="ExternalOutput").ap()
    # cs chunk rows: 0/1 = g1 h0/h1; 2/3 = g2 h0/h1; 4/5 = g3 h0/h1;
    # 6 = cs4a (m<4); 7 = cs4b (m>=4); 8/9 = g0 triangle h0/h1 (slots
    # 0:128 of row 8 are garbage, host zeroes them).
    cso = nc.dram_tensor("cs", (10, 512), f32, kind="ExternalOutput").ap()

    with tile.TileContext(nc) as tc:
        _keep = []

        def Tt(shape, dtype, name):
            t, free = tc.tile(shape, dtype, name=name)
            _keep.append(free)
            return t

        wts0 = Tt([128, 2, LOCAL], f8, "wts0")
        wts14 = Tt([128, 4, 2, LOCAL], f8, "wts14")
        exp_sb = Tt([128, 8, 1024], bf16, "expsb")   # slot k%8
        acc = Tt([128, 40], f32, "acc")              # [:, blk*8+m]
        cs_sb = Tt([128, 4, 512], f32, "cssb")
        ones = Tt([128, 1], bf16, "ones")
        nc.vector.memset(ones, 1.0)

        def wg(g):
            return wts0 if g == 0 else wts14[:, g - 1]

        widths = []
        for blk in range(5):
            for m in range(8):
                if blk == 0:
                    widths.append(1024 - m * 128)
                elif blk == 4:
                    widths.append(512)
                else:
                    widths.append(1024)
        engs = _assign_engines(widths)

        with tc.tile_pool(name="mtp", bufs=3, space="PSUM") as pmt, \
                tc.tile_pool(name="csp", bufs=2, space="PSUM") as pcs:

            nc.sync.dma_start(wts0, wt0)
            # wts14's dma_start is emitted AFTER blk0's body: DMA-completion
            # waits lump by emission order, so blk0's matmuls then gate on
            # wt0's ticks only (measured: first matmul 9.7us -> ~7.5us)
            cs_cur = {}
            pending_chains = None
            for blk in range(5):
                if blk == 1:
                    nc.sync.dma_start(wts14, wt14)
                for m in range(8):
                    mt = pmt.tile([128, 1024], f32, name=f"mt{blk}_{m}",
                                  tag="ps")
                    lhs = wg(0)[:, :, m * 128:(m + 1) * 128]
                    if blk == 0:
                        lo = m * 128
                        if m < 4:
                            nc.tensor.matmul(mt[:, lo:512], lhs,
                                             wg(0)[:, :, lo:512],
                                             start=True, stop=True,
                                             perf_mode=PM.DoubleRow)
                            nc.tensor.matmul(mt[:, 512:1024], lhs,
                                             wg(0)[:, :, 512:1024],
                                             start=True, stop=True,
                                             perf_mode=PM.DoubleRow)
                        else:
                            nc.tensor.matmul(mt[:, lo:1024], lhs,
                                             wg(0)[:, :, lo:1024],
                                             start=True, stop=True,
                                             perf_mode=PM.DoubleRow)
                        c0, c1 = lo, 1024
                    elif blk == 4:
                        c0, c1 = (0, 512) if m < 4 else (512, 1024)
                        nc.tensor.matmul(mt[:, c0:c1], lhs,
                                         wg(4)[:, :, c0:c1],
                                         start=True, stop=True,
                                         perf_mode=PM.DoubleRow)
                    else:
                        for c in range(2):
                            nc.tensor.matmul(mt[:, c * 512:(c + 1) * 512],
                                             lhs,
                                             wg(blk)[:, :, c * 512:(c + 1) * 512],
                                             start=True, stop=True,
                                             perf_mode=PM.DoubleRow)
                        c0, c1 = 0, 1024

                    k = blk * 8 + m
                    slot = exp_sb[:, k % 8, :]
                    if engs[k] == "A":
                        nc.scalar.activation(slot[:, c0:c1], mt[:, c0:c1],
                                             AF.Exp,
                                             accum_out=acc[:, k:k + 1])
                    else:
                        nc.vector.tensor_scalar(
                            slot[:, c0:c1].bitcast(i16), mt[:, c0:c1],
                            S_EXP, B_EXP, ALU.mult, ALU.add)
                        nc.vector.tensor_reduce(acc[:, k:k + 1],
                                                slot[:, c0:c1],
                                                AX.X, ALU.add)

                    # --- colsum chains, <=3 chunks per psum tile at
                    # partitions 0/32/64.  T3: blk0 h0/h1.  T1: g1 h0/h1 +
                    # g2 h0.  T2: g2 h1 + g3 h0/h1.  T4: cs4a/cs4b.
                    # Chain EMISSION is deferred by one tile so the PE
                    # queue runs [mains m, mains m+1, chains m, ...]: the
                    # chain matmuls wait on exp(m), and queueing them
                    # after the next tile's mains removes ~1us PE stalls
                    # per tile (measured chain waits 0.9-1.2us).
                    def emit_chains(blk=blk, m=m, slot=slot):
                        if blk == 0:
                            cst = cs_cur["T3"]
                            lo0 = (m + 1) * 128
                            if lo0 < 512:
                                nc.tensor.matmul(cst[0:1, lo0:512], ones,
                                                 slot[:, lo0:512],
                                                 start=(m == 0),
                                                 stop=(m == 2),
                                                 skip_group_check=True)
                            lo1 = max(lo0, 512)
                            if lo1 < 1024:
                                nc.tensor.matmul(cst[32:33, lo1 - 512:512],
                                                 ones, slot[:, lo1:1024],
                                                 start=(m == 0),
                                                 stop=(m == 6),
                                                 skip_group_check=True)
                            if m == 7:
                                nc.vector.tensor_copy(cs_sb[0:33, 2, :],
                                                      cst[0:33, :])
                                nc.gpsimd.dma_start(cso[8:9, :],
                                                    cs_sb[0:1, 2, :])
                                nc.gpsimd.dma_start(cso[9:10, :],
                                                    cs_sb[32:33, 2, :])
                        elif blk == 4:
                            cst = cs_cur["T4"]
                            if m < 4:
                                nc.tensor.matmul(cst[0:1, :], ones,
                                                 slot[:, 0:512],
                                                 start=(m == 0),
                                                 stop=(m == 3))
                            else:
                                nc.tensor.matmul(cst[32:33, :], ones,
                                                 slot[:, 512:1024],
                                                 start=(m == 4),
                                                 stop=(m == 7))
                            if m == 3:
                                nc.vector.tensor_copy(cs_sb[0:1, 3, :],
                                                      cst[0:1, :])
                                nc.sync.dma_start(cso[6:7, :],
                                                  cs_sb[0:1, 3, :])
                            if m == 7:
                                # drain-side: ACT is idle after its last
                                # exp while DVE still runs final reduces;
                                # staging the last chunk on ACT pulls the
                                # final cso DMA off DVE's tail
                                nc.scalar.copy(cs_sb[32:33, 3, :],
                                               cst[32:33, :])
                                nc.sync.dma_start(cso[7:8, :],
                                                  cs_sb[32:33, 3, :])
                        else:
                            for h in range(2):
                                ch = 2 * (blk - 1) + h
                                cstg = (cs_cur["T1"] if ch < 3
                                        else cs_cur["T2"])
                                pb = (ch % 3) * 32
                                nc.tensor.matmul(
                                    cstg[pb:pb + 1, :], ones,
                                    slot[:, h * 512:(h + 1) * 512],
                                    start=(m == 0), stop=(m == 7))
                            if blk == 2 and m == 7:
                                nc.vector.tensor_copy(cs_sb[0:65, 0, :],
                                                      cs_cur["T1"][0:65, :])
                                for j in range(3):
                                    nc.gpsimd.dma_start(
                                        cso[j:j + 1, :],
                                        cs_sb[j * 32:j * 32 + 1, 0, :])
                            if blk == 3 and m == 7:
                                nc.vector.tensor_copy(cs_sb[0:65, 1, :],
                                                      cs_cur["T2"][0:65, :])
                                for j in range(3):
                                    nc.gpsimd.dma_start(
                                        cso[3 + j:4 + j, :],
                                        cs_sb[j * 32:j * 32 + 1, 1, :])

                    # allocate chain psum tiles at the tile where their
                    # first chain matmul will actually be emitted
                    if blk == 0 and m == 0:
                        cs_cur["T3"] = pcs.tile([128, 512], f32,
                                                name="csT3", tag="cs")
                    if blk == 1 and m == 0:
                        cs_cur["T1"] = pcs.tile([128, 512], f32,
                                                name="csT1", tag="cs")
                    if blk == 2 and m == 0:
                        cs_cur["T2"] = pcs.tile([128, 512], f32,
                                                name="csT2", tag="cs")
                    if blk == 4 and m == 0:
                        cs_cur["T4"] = pcs.tile([128, 512], f32,
                                                name="csT4", tag="cs")
                    if pending_chains is not None:
                        pending_chains()
                    pending_chains = emit_chains

            pending_chains()
            nc.sync.dma_start(outd, acc)

        for free in reversed(_keep):
            free()

    nc.compile()
    return nc


def _get_nc():
    if "nc" not in _NC_CACHE:
        _NC_CACHE["nc"] = _build_program()
    return _NC_CACHE["nc"]


def _prep(emb_cat):
    emb = np.asarray(emb_cat, dtype=np.float32).astype(np.float64)
    nrm = np.maximum(np.sqrt((emb * emb).sum(1, keepdims=True)), 1e-12)
    z = emb / nrm
    w8 = (z / np.sqrt(T)).astype(np.float32).astype(ml_dtypes.float8_e4m3)
    # wt8[b, p, k, r] = w8[b*1024 + r, k*128 + p]
    wt8 = np.ascontiguousarray(
        w8.reshape(NCORES, LOCAL, 2, 128).transpose(0, 3, 2, 1))
    return z, w8, wt8


def _core_maps(wt8, c):
    gs = [wt8[(c + g) % NCORES] for g in range(5)]
    if c >= 4:
        # swap column halves of the pair block so (c, c+4) cover all
        # four quadrants between them
        g4 = gs[4]
        gs[4] = np.concatenate([g4[:, :, 512:], g4[:, :, :512]], axis=2)
    return {"wt0": np.ascontiguousarray(gs[0]),
            "wt14": np.ascontiguousarray(
                np.stack(gs[1:]).transpose(1, 0, 2, 3))}


def make_in_maps(emb_cat):
    _, _, wt8 = _prep(emb_cat)
    return [_core_maps(wt8, c) for c in range(NCORES)]


def kernel(emb_cat):
    from concourse import bass_utils

    emb_cat = np.ascontiguousarray(np.asarray(emb_cat, dtype=np.float32))
    assert emb_cat.shape == (N, D)
    nc = _get_nc()
    z, w8, wt8 = _prep(emb_cat)
    in_maps = [_core_maps(wt8, c) for c in range(NCORES)]
    res = bass_utils.run_bass_kernel_spmd(nc, in_maps,
                                          core_ids=list(range(NCORES)))

    rows = np.zeros((NCORES, LOCAL))
    cols = np.zeros((NCORES, 3, LOCAL))
    g0cs = np.zeros((NCORES, LOCAL))
    cs4a = np.zeros((NCORES, 512))
    cs4b = np.zeros((NCORES, 512))
    for c, r in enumerate(res.results):
        a = np.asarray(r["acc"], dtype=np.float64)     # [128, 40]
        rows[c] = a.reshape(128, 5, 8).sum(1).T.reshape(LOCAL)
        csm = np.asarray(r["cs"], dtype=np.float64)    # [10, 512]
        for g in (1, 2, 3):
            cols[c, g - 1] = np.concatenate(
                [csm[2 * (g - 1)], csm[2 * g - 1]])
        cs4a[c] = csm[6]
        cs4b[c] = csm[7]
        g0cs[c] = np.concatenate([csm[8], csm[9]])
        g0cs[c, :128] = 0.0

    pos = (z * np.roll(z, -B, axis=0)).sum(1) / T
    selfterm = np.exp((w8.astype(np.float64) ** 2).sum(1))

    total = 0.0
    for c in range(NCORES):
        gidx = (np.arange(LOCAL) + c * LOCAL) % N
        q = (c + 4) % 8
        g4 = np.empty(LOCAL)
        if c < 4:
            g4[:512] = cs4b[q]
            g4[512:] = cs4a[q]
        else:
            g4[:512] = cs4a[q]
            g4[512:] = cs4b[q]
        denom = (rows[c] + g0cs[c] + g4 - selfterm[gidx]
                 + cols[(c + 5) % 8][2]
                 + cols[(c + 6) % 8][1]
                 + cols[(c + 7) % 8][0])
        total += (np.log(denom) - pos[gidx]).sum()
    return np.float32(total / B)
